# revision 10
# baseline (speedup 1.0000x reference)
"""Trainium2 Bass kernel: dense transformer encoder layer (pre-norm, RoPE, GELU FFN).

Sharding: 8 cores, zero-communication. Core c handles batch b=c//2 and query
half q=c%2 (1024 of 2048 rows). K/V projections over the full sequence are
duplicated between the two cores of a batch. Everything on-chip is computed in
transposed layout [d, t]; host pre-transposes weights/inputs and un-transposes
the output.

Tricks:
  - LN affine (g, be) folded into the following projection weights on host.
  - RoPE pair-rotation done as q' = cos*qA + sin*(J @ qA) with J a constant
    128x128 pair-swap matrix applied on the TensorEngine.
  - Softmax row-sums come from a ones-column appended to V (PV matmul output
    row 64 = sum over keys); no separate reduction pass.
  - Per-row biases in row-layout (V) added via a K=1 ones-outer-product matmul
    into the same PSUM accumulation group.
"""

import numpy as np
import ml_dtypes

B, T, D, NH, DF = 4, 2048, 1024, 16, 4096
P = 128
TQ = T // 2            # per-core query rows
DO = D // P            # 8 d-tiles
KT = T // P            # 16 key tiles
DFT = DF // P          # 32 ff tiles
NQ = TQ // 512         # 2 query slices of 512
NT = T // 512          # 4 key-time slices of 512
DK = D // NH           # 64
LN_EPS = 1e-5
ROPE_HZ = 10000.0
N_CORES = 8

_CACHE = {}
# extra kwargs for run_bass_kernel_spmd (e.g. trace=True from a test harness)
RUN_KWARGS = {}


def _build_program():
    import concourse.bass as bass  # noqa: F401
    import concourse.bacc as bacc
    import concourse.mybir as mybir
    import concourse.tile as tile

    F32 = mybir.dt.float32
    BF16 = mybir.dt.bfloat16
    AF = mybir.ActivationFunctionType
    OP = mybir.AluOpType

    nc = bacc.Bacc("TRN2", target_bir_lowering=False)

    # ---------------- DRAM I/O ----------------
    xT = nc.dram_tensor("xT", [D, T], F32, kind="ExternalInput")
    cosm = nc.dram_tensor("cosm", [P, T], BF16, kind="ExternalInput")
    sinm = nc.dram_tensor("sinm", [P, T], BF16, kind="ExternalInput")
    wqT = nc.dram_tensor("wqT", [D, D], BF16, kind="ExternalInput")
    wkT = nc.dram_tensor("wkT", [D, D], BF16, kind="ExternalInput")
    wvT = nc.dram_tensor("wvT", [D, D], BF16, kind="ExternalInput")
    woT = nc.dram_tensor("woT", [D, D], BF16, kind="ExternalInput")
    jTd = nc.dram_tensor("jT", [P, P], BF16, kind="ExternalInput")
    w1T = nc.dram_tensor("w1T", [D, DF], BF16, kind="ExternalInput")
    w2T = nc.dram_tensor("w2T", [DF, D], BF16, kind="ExternalInput")
    bqd = nc.dram_tensor("bq", [P, DO], F32, kind="ExternalInput")
    bkd = nc.dram_tensor("bk", [P, DO], F32, kind="ExternalInput")
    bod = nc.dram_tensor("bo", [P, DO], F32, kind="ExternalInput")
    b2d = nc.dram_tensor("b2", [P, DO], F32, kind="ExternalInput")
    b1d = nc.dram_tensor("b1", [P, DFT], F32, kind="ExternalInput")
    bvd = nc.dram_tensor("bv", [1, D], F32, kind="ExternalInput")
    outT = nc.dram_tensor("outT", [D, TQ], F32, kind="ExternalOutput")

    wqT_r = wqT.rearrange("(o p) d -> p o d", p=P)
    wkT_r = wkT.rearrange("(o p) d -> p o d", p=P)
    wvT_r = wvT.rearrange("(o p) d -> p o d", p=P)
    woT_r = woT.rearrange("(o p) d -> p o d", p=P)
    w1T_r = w1T.rearrange("(o p) d -> p o d", p=P)

    with tile.TileContext(nc) as tc:
        pconst = tc.alloc_tile_pool(name="pconst", bufs=1)
        pW = tc.alloc_tile_pool(name="pW", bufs=1)
        pDR = tc.alloc_tile_pool(name="pDR", bufs=1, space="DRAM")
        pAT = tc.alloc_tile_pool(name="pAT", bufs=1)

        # constants
        ones_col = pconst.tile([P, 1], F32)
        nc.vector.memset(ones_col, 1.0)
        ones_row = pconst.tile([1, P], F32)
        nc.vector.memset(ones_row, 1.0)
        jT_sb = pconst.tile([P, P], BF16)
        nc.sync.dma_start(out=jT_sb, in_=jTd[:, :])
        bq_sb = pconst.tile([P, DO], F32)
        nc.sync.dma_start(out=bq_sb, in_=bqd[:, :])
        bk_sb = pconst.tile([P, DO], F32)
        nc.sync.dma_start(out=bk_sb, in_=bkd[:, :])
        bo_sb = pconst.tile([P, DO], F32)
        nc.sync.dma_start(out=bo_sb, in_=bod[:, :])
        b2_sb = pconst.tile([P, DO], F32)
        nc.sync.dma_start(out=b2_sb, in_=b2d[:, :])
        b1_sb = pconst.tile([P, DFT], F32)
        nc.sync.dma_start(out=b1_sb, in_=b1d[:, :])
        bv_sb = pconst.tile([1, D], F32)
        nc.sync.dma_start(out=bv_sb, in_=bvd[:, :])
        eps_sb = pconst.tile([P, 1], F32)
        nc.vector.memset(eps_sb, LN_EPS)

        psM = tc.alloc_tile_pool(name="psM", bufs=1, space="PSUM")
        pH = tc.alloc_tile_pool(name="pH", bufs=1)
        attnT = pAT.tile([P, DO, TQ], BF16)
        h_sb = pH.tile([P, DO, T], BF16)

        inv1_d = pDR.tile([1, T], F32)
        nm1_d = pDR.tile([1, T], F32)
        inv2_d = pDR.tile([1, TQ], F32)
        nm2_d = pDR.tile([1, TQ], F32)

        # ================= LN1 (transposed layout; stats over partitions via PE) =================
        pA = tc.alloc_tile_pool(name="pA", bufs=1)
        st1 = psM.tile([P, 4, 512], F32, tag="sc", bufs=1, name="st1")
        ps_sum = [st1[0:1, i, :] for i in range(NT)]
        ps_sq = ([psM.tile([P, 512], F32, tag="so", bufs=2, name=f"ps_sqa{i}")[0:1, :] for i in range(2)]
                 + [psM.tile([P, 512], F32, tag="mm", bufs=2, name=f"ps_sqb{i}")[0:1, :] for i in range(2)])

        for o in range(DO):
            xs = pA.tile([P, T], F32, tag="xs", bufs=3, name="xs")
            nc.sync.dma_start(out=xs, in_=xT[o * P:(o + 1) * P, :])
            xsq = pA.tile([P, T], F32, tag="xsq", bufs=2, name="xsq")
            nc.scalar.activation(out=xsq, in_=xs, func=AF.Square)
            for n in range(NT):
                s = slice(n * 512, (n + 1) * 512)
                nc.tensor.matmul(ps_sum[n], lhsT=ones_col[:, 0:1], rhs=xs[:, s],
                                 start=(o == 0), stop=(o == DO - 1))
                nc.tensor.matmul(ps_sq[n], lhsT=ones_col[:, 0:1], rhs=xsq[:, s],
                                 start=(o == 0), stop=(o == DO - 1))

        # stats rows: separate tiles, everything at partition base 0
        m_r = pA.tile([1, T], F32, tag="row", bufs=3, name="m_r")
        e_r = pA.tile([1, T], F32, tag="row", bufs=3, name="e_r")
        for n in range(NT):
            s = slice(n * 512, (n + 1) * 512)
            nc.vector.tensor_scalar_mul(m_r[0:1, s], ps_sum[n], 1.0 / D)
            nc.vector.tensor_scalar_mul(e_r[0:1, s], ps_sq[n], 1.0 / D)
        v_r = pA.tile([1, T], F32, tag="row", bufs=3, name="v_r")
        nc.vector.tensor_mul(v_r, m_r, m_r)
        nc.vector.tensor_sub(v_r, e_r, v_r)
        nc.scalar.activation(out=v_r, in_=v_r, func=AF.Sqrt, bias=eps_sb[0:1, 0:1])
        i_r = pA.tile([1, T], F32, tag="row", bufs=3, name="i_r")
        nc.vector.reciprocal(i_r, v_r)
        n_r = pA.tile([1, T], F32, tag="row", bufs=3, name="n_r")
        nc.vector.tensor_mul(n_r, m_r, i_r)
        nc.vector.tensor_scalar_mul(n_r, n_r, -1.0)
        nc.sync.dma_start(out=inv1_d, in_=i_r[0:1, :])
        nc.sync.dma_start(out=nm1_d, in_=n_r[0:1, :])
        inv_b = pA.tile([P, T], F32, name="inv_b")
        nc.sync.dma_start(out=inv_b, in_=inv1_d[0:1, :].to_broadcast((P, T)))
        nm_b = pA.tile([P, T], F32, name="nm_b")
        nc.sync.dma_start(out=nm_b, in_=nm1_d[0:1, :].to_broadcast((P, T)))

        for o in range(DO):
            xs = pA.tile([P, T], F32, tag="xs", bufs=3, name="xs2")
            nc.sync.dma_start(out=xs, in_=xT[o * P:(o + 1) * P, :])
            tmp = pA.tile([P, T], F32, tag="lntmp", bufs=2, name="lntmp")
            nc.vector.tensor_mul(tmp, xs, inv_b)
            nc.vector.tensor_add(h_sb[:, o, :], tmp, nm_b)
        pA.release()

        # ================= QKV projections + RoPE =================
        pWV = tc.alloc_tile_pool(name="pWV", bufs=1)
        pATT = tc.alloc_tile_pool(name="pATT", bufs=1)

        cos_sb = pWV.tile([P, T], BF16, name="cos_sb")
        nc.sync.dma_start(out=cos_sb, in_=cosm[:, :])
        sin_sb = pWV.tile([P, T], BF16, name="sin_sb")
        nc.sync.dma_start(out=sin_sb, in_=sinm[:, :])

        qT_sb = pATT.tile([P, DO, TQ], BF16, name="qT_sb")
        kT_sb = pATT.tile([P, DO, T], BF16, name="kT_sb")
        v_sb = pATT.tile([P, KT, NH, DK + 1], BF16, name="v_sb")
        nc.vector.memset(v_sb[:, :, :, DK:DK + 1], 1.0)

        # V (row layout [t, dv]) with bias via ones-outer matmul
        for dn in range(2):
            wv_h = pWV.tile([P, DO, 512], BF16, tag="wvh", bufs=1, name="wv_h")
            nc.sync.dma_start(out=wv_h, in_=wvT_r[:, :, dn * 512:(dn + 1) * 512])
            for tt in range(KT):
                psv = psM.tile([P, 512], F32, tag="mm", bufs=2, name="psv")
                for o in range(DO):
                    nc.tensor.matmul(psv, lhsT=h_sb[:, o, tt * P:(tt + 1) * P], rhs=wv_h[:, o, :],
                                     start=(o == 0), stop=False)
                nc.tensor.matmul(psv, lhsT=ones_row[0:1, 0:P], rhs=bv_sb[0:1, dn * 512:(dn + 1) * 512],
                                 start=False, stop=True)
                nc.vector.tensor_copy(out=v_sb[:, tt, dn * 8:(dn + 1) * 8, 0:DK],
                                      in_=psv.rearrange("p (a b) -> p a b", a=8))

        # Q^T and K^T (transposed layout), with rotary applied
        def proj_rot(j, n, w_t, b_col, dst, dst_slice):
            s = slice(n * 512, (n + 1) * 512)
            ps = psM.tile([P, 512], F32, tag="mm", bufs=2, name="psqk")
            for o in range(DO):
                nc.tensor.matmul(ps, lhsT=w_t[:, o, :], rhs=h_sb[:, o, s],
                                 start=(o == 0), stop=(o == DO - 1))
            qa = pATT.tile([P, 512], BF16, tag="qa", bufs=3, name="qa")
            nc.scalar.activation(out=qa, in_=ps, func=AF.Identity, bias=b_col)
            psb = psM.tile([P, 512], F32, tag="mm", bufs=2, name="psjb")
            nc.tensor.matmul(psb, lhsT=jT_sb, rhs=qa, start=True, stop=True)
            rt1 = pATT.tile([P, 512], F32, tag="rt1", bufs=2, name="rt1")
            nc.vector.tensor_mul(rt1, qa, cos_sb[:, s])
            rt2 = pATT.tile([P, 512], F32, tag="rt2", bufs=2, name="rt2")
            nc.vector.tensor_tensor(rt2, psb, sin_sb[:, s], op=OP.mult)
            nc.vector.tensor_add(dst[:, j, dst_slice], rt1, rt2)

        for j in range(DO):
            wq_t = pW.tile([P, DO, P], BF16, tag="wchunk", bufs=3, name="wq_t")
            nc.sync.dma_start(out=wq_t, in_=wqT_r[:, :, j * P:(j + 1) * P])
            for n in range(NQ):
                proj_rot(j, n, wq_t, bq_sb[:, j:j + 1], qT_sb, slice(n * 512, (n + 1) * 512))
            wk_t = pW.tile([P, DO, P], BF16, tag="wchunk", bufs=3, name="wk_t")
            nc.sync.dma_start(out=wk_t, in_=wkT_r[:, :, j * P:(j + 1) * P])
            for n in range(NT):
                proj_rot(j, n, wk_t, bk_sb[:, j:j + 1], kT_sb, slice(n * 512, (n + 1) * 512))

        # ================= attention =================
        for h in range(NH):
            jh, off = h // 2, DK * (h % 2)
            for n in range(NQ):
                qs = slice(n * 512, (n + 1) * 512)
                pso = psM.tile([P, 512], F32, tag="so", bufs=2, name="pso")
                for g in range(KT // 4):
                    sc = psM.tile([P, 4, 512], F32, tag="sc", bufs=1, name="sc")
                    for kk in range(4):
                        kt = g * 4 + kk
                        nc.tensor.matmul(sc[:, kk, :],
                                         lhsT=kT_sb[off:off + DK, jh, kt * P:(kt + 1) * P],
                                         rhs=qT_sb[off:off + DK, jh, qs],
                                         start=True, stop=True)
                    pt = pATT.tile([P, 4, 512], BF16, tag="pt", bufs=2, name="pt")
                    nc.scalar.activation(out=pt, in_=sc, func=AF.Exp, scale=0.125)
                    for kk in range(4):
                        kt = g * 4 + kk
                        nc.tensor.matmul(pso[0:DK + 1, :], lhsT=v_sb[:, kt, h, :], rhs=pt[:, kk, :],
                                         start=(kt == 0), stop=(kt == KT - 1))
                rec = pATT.tile([1, 512], F32, tag="rec", bufs=2, name="rec")
                nc.vector.reciprocal(rec, pso[DK:DK + 1, :])
                prb = psM.tile([P, 512], F32, tag="mm", bufs=2, name="prb")
                nc.tensor.matmul(prb[0:DK, :], lhsT=ones_row[0:1, 0:DK], rhs=rec[0:1, :],
                                 start=True, stop=True)
                recb = pATT.tile([DK, 512], F32, tag="recb", bufs=2, name="recb")
                nc.vector.tensor_copy(out=recb, in_=prb[0:DK, :])
                nc.vector.tensor_mul(attnT[off:off + DK, jh, qs], pso[0:DK, :], recb[0:DK, :])
        pATT.release()
        pWV.release()
        pH.release()

        # ================= out-projection + residual =================
        pP = tc.alloc_tile_pool(name="pP", bufs=1)
        y1 = pP.tile([P, DO, TQ], F32, name="y1")
        for j in range(DO):
            wo_t = pW.tile([P, DO, P], BF16, tag="wchunk", bufs=3, name="wo_t")
            nc.sync.dma_start(out=wo_t, in_=woT_r[:, :, j * P:(j + 1) * P])
            xr = pW.tile([P, TQ], F32, tag="xres", bufs=2, name="xr")
            nc.sync.dma_start(out=xr, in_=xT[j * P:(j + 1) * P, 0:TQ])
            for n in range(NQ):
                qs = slice(n * 512, (n + 1) * 512)
                psy = psM.tile([P, 512], F32, tag="mm", bufs=2, name="psy")
                for o in range(DO):
                    nc.tensor.matmul(psy, lhsT=wo_t[:, o, :], rhs=attnT[:, o, qs],
                                     start=(o == 0), stop=(o == DO - 1))
                nc.vector.scalar_tensor_tensor(out=y1[:, j, qs], in0=psy, scalar=bo_sb[:, j:j + 1],
                                               in1=xr[:, qs], op0=OP.add, op1=OP.add)
        # ================= LN2 =================
        pF = tc.alloc_tile_pool(name="pF", bufs=1)
        st2 = psM.tile([P, 4, 512], F32, tag="sc", bufs=1, name="st2")
        for o in range(DO):
            xsq2 = pF.tile([P, TQ], F32, tag="xsq2", bufs=1, name="xsq2")
            nc.scalar.activation(out=xsq2, in_=y1[:, o, :], func=AF.Square)
            for n in range(NQ):
                qs = slice(n * 512, (n + 1) * 512)
                nc.tensor.matmul(st2[0:1, n, :], lhsT=ones_col[:, 0:1], rhs=y1[:, o, qs],
                                 start=(o == 0), stop=(o == DO - 1))
                nc.tensor.matmul(st2[0:1, 2 + n, :], lhsT=ones_col[:, 0:1], rhs=xsq2[:, qs],
                                 start=(o == 0), stop=(o == DO - 1))
        m2_r = pF.tile([1, TQ], F32, tag="r2", bufs=3, name="m2_r")
        e2_r = pF.tile([1, TQ], F32, tag="r2", bufs=3, name="e2_r")
        for n in range(NQ):
            qs = slice(n * 512, (n + 1) * 512)
            nc.vector.tensor_scalar_mul(m2_r[0:1, qs], st2[0:1, n, :], 1.0 / D)
            nc.vector.tensor_scalar_mul(e2_r[0:1, qs], st2[0:1, 2 + n, :], 1.0 / D)
        v2_r = pF.tile([1, TQ], F32, tag="r2", bufs=3, name="v2_r")
        nc.vector.tensor_mul(v2_r, m2_r, m2_r)
        nc.vector.tensor_sub(v2_r, e2_r, v2_r)
        nc.scalar.activation(out=v2_r, in_=v2_r, func=AF.Sqrt, bias=eps_sb[0:1, 0:1])
        i2_r = pF.tile([1, TQ], F32, tag="r2", bufs=3, name="i2_r")
        nc.vector.reciprocal(i2_r, v2_r)
        n2_r = pF.tile([1, TQ], F32, tag="r2", bufs=3, name="n2_r")
        nc.vector.tensor_mul(n2_r, m2_r, i2_r)
        nc.vector.tensor_scalar_mul(n2_r, n2_r, -1.0)
        nc.sync.dma_start(out=inv2_d, in_=i2_r[0:1, :])
        nc.sync.dma_start(out=nm2_d, in_=n2_r[0:1, :])
        inv2_b = pF.tile([P, TQ], F32, name="inv2_b")
        nc.sync.dma_start(out=inv2_b, in_=inv2_d[0:1, :].to_broadcast((P, TQ)))
        nm2_b = pF.tile([P, TQ], F32, name="nm2_b")
        nc.sync.dma_start(out=nm2_b, in_=nm2_d[0:1, :].to_broadcast((P, TQ)))
        h2 = pF.tile([P, DO, TQ], BF16, name="h2")
        for o in range(DO):
            t2 = pF.tile([P, TQ], F32, tag="lntmp2", bufs=1, name="t2")
            nc.vector.tensor_mul(t2, y1[:, o, :], inv2_b)
            nc.vector.tensor_add(h2[:, o, :], t2, nm2_b)

        # ================= FFN =================
        gsb = [pF.tile([P, DFT, 512], BF16, tag="g", bufs=2, name=f"gsb{n}") for n in range(NQ)]
        for f in range(DFT):
            w1_t = pW.tile([P, DO, P], BF16, tag="wchunk", bufs=3, name="w1_t")
            nc.sync.dma_start(out=w1_t, in_=w1T_r[:, :, f * P:(f + 1) * P])
            for n in range(NQ):
                qs = slice(n * 512, (n + 1) * 512)
                psa = psM.tile([P, 512], F32, tag="mm", bufs=2, name="psa")
                for o in range(DO):
                    nc.tensor.matmul(psa, lhsT=w1_t[:, o, :], rhs=h2[:, o, qs],
                                     start=(o == 0), stop=(o == DO - 1))
                nc.scalar.activation(out=gsb[n][:, f, :], in_=psa, func=AF.Gelu,
                                     bias=b1_sb[:, f:f + 1])
        for j in range(DO):
            psz = [psM.tile([P, 512], F32, tag="so", bufs=2, name=f"psz{n}") for n in range(NQ)]
            for f in range(DFT):
                w2_t = pW.tile([P, P], BF16, tag="w2c", bufs=4, name="w2_t")
                nc.sync.dma_start(out=w2_t, in_=w2T[f * P:(f + 1) * P, j * P:(j + 1) * P])
                for n in range(NQ):
                    nc.tensor.matmul(psz[n], lhsT=w2_t, rhs=gsb[n][:, f, :],
                                     start=(f == 0), stop=(f == DFT - 1))
            for n in range(NQ):
                qs = slice(n * 512, (n + 1) * 512)
                osb = pF.tile([P, 512], F32, tag="osb", bufs=2, name="osb")
                nc.vector.scalar_tensor_tensor(out=osb, in0=psz[n], scalar=b2_sb[:, j:j + 1],
                                               in1=y1[:, j, qs], op0=OP.add, op1=OP.add)
                nc.sync.dma_start(out=outT[j * P:(j + 1) * P, qs], in_=osb)

        pF.release()
        pP.release()
        pH_done = True  # pH released after attention
        psM.release()
        pAT.release()
        pDR.release()
        pW.release()
        pconst.release()

    nc.finalize()
    return nc


def _prep_inputs(inputs):
    f32 = np.float32
    bf16 = ml_dtypes.bfloat16
    x = np.asarray(inputs["x"], dtype=f32)
    Wq = np.asarray(inputs["Wq"], dtype=f32)
    bq = np.asarray(inputs["bq"], dtype=f32)
    Wk = np.asarray(inputs["Wk"], dtype=f32)
    bk = np.asarray(inputs["bk"], dtype=f32)
    Wv = np.asarray(inputs["Wv"], dtype=f32)
    bv = np.asarray(inputs["bv"], dtype=f32)
    Wo = np.asarray(inputs["Wo"], dtype=f32)
    bo = np.asarray(inputs["bo"], dtype=f32)
    W1 = np.asarray(inputs["W1"], dtype=f32)
    b1 = np.asarray(inputs["b1"], dtype=f32)
    W2 = np.asarray(inputs["W2"], dtype=f32)
    b2 = np.asarray(inputs["b2"], dtype=f32)
    g1 = np.asarray(inputs["g1"], dtype=f32)
    be1 = np.asarray(inputs["be1"], dtype=f32)
    g2 = np.asarray(inputs["g2"], dtype=f32)
    be2 = np.asarray(inputs["be2"], dtype=f32)

    # fold LN affines into the consuming projections
    Wq_f, bq_f = Wq * g1[None, :], bq + Wq @ be1
    Wk_f, bk_f = Wk * g1[None, :], bk + Wk @ be1
    Wv_f, bv_f = Wv * g1[None, :], bv + Wv @ be1
    W1_f, b1_f = W1 * g2[None, :], b1 + W1 @ be2

    def t_bf16(w):
        return np.ascontiguousarray(w.T).astype(bf16)

    def pack(b):
        return np.ascontiguousarray(b.reshape(-1, P).T).astype(f32)

    J = np.zeros((P, P), dtype=f32)
    for i in range(0, P, 2):
        J[i, i + 1] = -1.0
        J[i + 1, i] = 1.0

    shared = {
        "wqT": t_bf16(Wq_f), "wkT": t_bf16(Wk_f), "wvT": t_bf16(Wv_f),
        "woT": t_bf16(Wo), "w1T": t_bf16(W1_f), "w2T": t_bf16(W2),
        "jT": np.ascontiguousarray(J.T).astype(bf16),
        "bq": pack(bq_f), "bk": pack(bk_f), "bo": pack(bo), "b2": pack(b2),
        "b1": pack(b1_f), "bv": np.ascontiguousarray(bv_f[None, :]),
    }

    theta = 1.0 / (ROPE_HZ ** (np.arange(0, DK, 2, dtype=np.float64) / DK))  # [32]
    row_idx = (np.arange(P) % DK) // 2

    in_maps = []
    for c in range(N_CORES):
        b, half = c // 2, c % 2
        xTb = x[b].T  # [D, T]
        own = slice(half * TQ, half * TQ + TQ)
        oth = slice((1 - half) * TQ, (1 - half) * TQ + TQ)
        xin = np.ascontiguousarray(np.concatenate([xTb[:, own], xTb[:, oth]], axis=1))
        pos = np.concatenate([np.arange(half * TQ, half * TQ + TQ),
                              np.arange((1 - half) * TQ, (1 - half) * TQ + TQ)])
        ang = theta[:, None] * pos[None, :]  # [32, T]
        cosm = np.ascontiguousarray(np.cos(ang)[row_idx]).astype(bf16)
        sinm = np.ascontiguousarray(np.sin(ang)[row_idx]).astype(bf16)
        m = dict(shared)
        m["xT"] = xin
        m["cosm"] = cosm
        m["sinm"] = sinm
        in_maps.append(m)
    return in_maps


def kernel(**inputs):
    from concourse.bass_utils import run_bass_kernel_spmd

    if "nc" not in _CACHE:
        _CACHE["nc"] = _build_program()
    nc = _CACHE["nc"]
    in_maps = _prep_inputs(inputs)
    res = run_bass_kernel_spmd(nc, in_maps, core_ids=list(range(N_CORES)), **RUN_KWARGS)
    _CACHE["last_results"] = res
    out = np.empty((B, T, D), dtype=np.float32)
    for c in range(N_CORES):
        b, half = c // 2, c % 2
        out[b, half * TQ:half * TQ + TQ, :] = np.asarray(res.results[c]["outT"]).T
    return out


# revision 11
# speedup vs baseline: 1.5124x; 1.5124x over previous
"""Trainium2 Bass kernel: dense transformer encoder layer (pre-norm, RoPE, GELU FFN).

Sharding: 8 cores, zero-communication. Core c handles batch b=c//2 and query
half q=c%2 (1024 of 2048 rows). K/V projections over the full sequence are
duplicated between the two cores of a batch. Everything on-chip is computed in
transposed layout [d, t]; host pre-transposes weights/inputs and un-transposes
the output.

Tricks:
  - LN affine (g, be) folded into the following projection weights on host.
  - RoPE pair-rotation done as q' = cos*qA + sin*(J @ qA) with J a constant
    128x128 pair-swap matrix applied on the TensorEngine.
  - Softmax row-sums come from a ones-column appended to V (PV matmul output
    row 64 = sum over keys); no separate reduction pass.
  - Per-row biases in row-layout (V) added via a K=1 ones-outer-product matmul
    into the same PSUM accumulation group.
"""

import numpy as np
import ml_dtypes

B, T, D, NH, DF = 4, 2048, 1024, 16, 4096
P = 128
TQ = T // 2            # per-core query rows
DO = D // P            # 8 d-tiles
KT = T // P            # 16 key tiles
DFT = DF // P          # 32 ff tiles
NQ = TQ // 512         # 2 query slices of 512
NT = T // 512          # 4 key-time slices of 512
DK = D // NH           # 64
LN_EPS = 1e-5
ROPE_HZ = 10000.0
N_CORES = 8

_CACHE = {}
# extra kwargs for run_bass_kernel_spmd (e.g. trace=True from a test harness)
RUN_KWARGS = {}


def _build_program():
    import concourse.bass as bass  # noqa: F401
    import concourse.bacc as bacc
    import concourse.mybir as mybir
    import concourse.tile as tile

    F32 = mybir.dt.float32
    BF16 = mybir.dt.bfloat16
    AF = mybir.ActivationFunctionType
    OP = mybir.AluOpType

    nc = bacc.Bacc("TRN2", target_bir_lowering=False)

    # ---------------- DRAM I/O ----------------
    xT = nc.dram_tensor("xT", [D, T], F32, kind="ExternalInput")
    cosm = nc.dram_tensor("cosm", [P, T], BF16, kind="ExternalInput")
    sinm = nc.dram_tensor("sinm", [P, T], BF16, kind="ExternalInput")
    wqT = nc.dram_tensor("wqT", [D, D], BF16, kind="ExternalInput")
    wkT = nc.dram_tensor("wkT", [D, D], BF16, kind="ExternalInput")
    wvT = nc.dram_tensor("wvT", [D, D], BF16, kind="ExternalInput")
    woT = nc.dram_tensor("woT", [D, D], BF16, kind="ExternalInput")
    jTd = nc.dram_tensor("jT", [P, P], BF16, kind="ExternalInput")
    w1T = nc.dram_tensor("w1T", [D, DF], BF16, kind="ExternalInput")
    w2T = nc.dram_tensor("w2T", [DF, D], BF16, kind="ExternalInput")
    bqd = nc.dram_tensor("bq", [P, DO], F32, kind="ExternalInput")
    bkd = nc.dram_tensor("bk", [P, DO], F32, kind="ExternalInput")
    bod = nc.dram_tensor("bo", [P, DO], F32, kind="ExternalInput")
    b2d = nc.dram_tensor("b2", [P, DO], F32, kind="ExternalInput")
    b1d = nc.dram_tensor("b1", [P, DFT], F32, kind="ExternalInput")
    bvd = nc.dram_tensor("bv", [1, D], F32, kind="ExternalInput")
    outT = nc.dram_tensor("outT", [D, TQ], F32, kind="ExternalOutput")

    wqT_r = wqT.rearrange("(o p) d -> p o d", p=P)
    wkT_r = wkT.rearrange("(o p) d -> p o d", p=P)
    wvT_r = wvT.rearrange("(o p) d -> p o d", p=P)
    woT_r = woT.rearrange("(o p) d -> p o d", p=P)
    w1T_r = w1T.rearrange("(o p) d -> p o d", p=P)

    with tile.TileContext(nc) as tc:
        pconst = tc.alloc_tile_pool(name="pconst", bufs=1)
        pW = tc.alloc_tile_pool(name="pW", bufs=1)
        pDR = tc.alloc_tile_pool(name="pDR", bufs=1, space="DRAM")
        pAT = tc.alloc_tile_pool(name="pAT", bufs=1)

        # constants
        ones_col = pconst.tile([P, 1], F32)
        nc.vector.memset(ones_col, 1.0)
        ones_row = pconst.tile([1, P], F32)
        nc.vector.memset(ones_row, 1.0)
        jT_sb = pconst.tile([P, P], BF16)
        nc.sync.dma_start(out=jT_sb, in_=jTd[:, :])
        bq_sb = pconst.tile([P, DO], F32)
        nc.sync.dma_start(out=bq_sb, in_=bqd[:, :])
        bk_sb = pconst.tile([P, DO], F32)
        nc.sync.dma_start(out=bk_sb, in_=bkd[:, :])
        bo_sb = pconst.tile([P, DO], F32)
        nc.sync.dma_start(out=bo_sb, in_=bod[:, :])
        b2_sb = pconst.tile([P, DO], F32)
        nc.sync.dma_start(out=b2_sb, in_=b2d[:, :])
        b1_sb = pconst.tile([P, DFT], F32)
        nc.sync.dma_start(out=b1_sb, in_=b1d[:, :])
        bv_sb = pconst.tile([1, D], F32)
        nc.sync.dma_start(out=bv_sb, in_=bvd[:, :])
        eps_sb = pconst.tile([P, 1], F32)
        nc.vector.memset(eps_sb, LN_EPS)
        ones_bf = pconst.tile([P, 1], BF16)
        nc.vector.memset(ones_bf, 1.0)

        psM = tc.alloc_tile_pool(name="psM", bufs=1, space="PSUM")
        pH = tc.alloc_tile_pool(name="pH", bufs=1)
        attnT = pAT.tile([P, DO, TQ], BF16)
        h_sb = pH.tile([P, DO, T], BF16)

        inv1_d = pDR.tile([1, T], F32)
        nm1_d = pDR.tile([1, T], F32)
        inv2_d = pDR.tile([1, TQ], F32)
        nm2_d = pDR.tile([1, TQ], F32)

        # ================= LN1 (transposed layout; stats over partitions via PE) =================
        pA = tc.alloc_tile_pool(name="pA", bufs=1)
        st1a = psM.tile([P, 2, 512], F32, tag="sc", bufs=2, name="st1a")
        st1b = psM.tile([P, 2, 512], F32, tag="sc", bufs=2, name="st1b")
        ps_sum = [st1a[0:1, 0, :], st1a[0:1, 1, :], st1b[0:1, 0, :], st1b[0:1, 1, :]]
        ps_sq = ([psM.tile([P, 512], F32, tag="so", bufs=2, name=f"ps_sqa{i}")[0:1, :] for i in range(2)]
                 + [psM.tile([P, 512], F32, tag="mm", bufs=2, name=f"ps_sqb{i}")[0:1, :] for i in range(2)])

        xb_sb = pA.tile([P, DO, T], BF16, name="xb_sb")
        for o in range(DO):
            xs = pA.tile([P, T], F32, tag="xs", bufs=2, name="xs")
            nc.sync.dma_start(out=xs, in_=xT[o * P:(o + 1) * P, :])
            nc.vector.tensor_copy(out=xb_sb[:, o, :], in_=xs)
            xsq = pA.tile([P, T], BF16, tag="xsq", bufs=2, name="xsq")
            nc.scalar.activation(out=xsq, in_=xs, func=AF.Square)
            for n in range(NT):
                s = slice(n * 512, (n + 1) * 512)
                nc.tensor.matmul(ps_sum[n], lhsT=ones_bf[:, 0:1], rhs=xb_sb[:, o, s],
                                 start=(o == 0), stop=(o == DO - 1))
                nc.tensor.matmul(ps_sq[n], lhsT=ones_bf[:, 0:1], rhs=xsq[:, s],
                                 start=(o == 0), stop=(o == DO - 1))

        # stats rows: separate tiles, everything at partition base 0
        m_r = pA.tile([1, T], F32, tag="row", bufs=3, name="m_r")
        e_r = pA.tile([1, T], F32, tag="row", bufs=3, name="e_r")
        for n in range(NT):
            s = slice(n * 512, (n + 1) * 512)
            nc.vector.tensor_scalar_mul(m_r[0:1, s], ps_sum[n], 1.0 / D)
            nc.vector.tensor_scalar_mul(e_r[0:1, s], ps_sq[n], 1.0 / D)
        v_r = pA.tile([1, T], F32, tag="row", bufs=3, name="v_r")
        nc.vector.tensor_mul(v_r, m_r, m_r)
        nc.vector.tensor_sub(v_r, e_r, v_r)
        nc.scalar.activation(out=v_r, in_=v_r, func=AF.Sqrt, bias=eps_sb[0:1, 0:1])
        i_r = pA.tile([1, T], F32, tag="row", bufs=3, name="i_r")
        nc.vector.reciprocal(i_r, v_r)
        n_r = pA.tile([1, T], F32, tag="row", bufs=3, name="n_r")
        nc.vector.tensor_mul(n_r, m_r, i_r)
        nc.vector.tensor_scalar_mul(n_r, n_r, -1.0)
        nc.sync.dma_start(out=inv1_d, in_=i_r[0:1, :])
        nc.sync.dma_start(out=nm1_d, in_=n_r[0:1, :])
        inv_b = pA.tile([P, T], F32, name="inv_b")
        nc.sync.dma_start(out=inv_b, in_=inv1_d[0:1, :].to_broadcast((P, T)))
        nm_b = pA.tile([P, T], F32, name="nm_b")
        nc.sync.dma_start(out=nm_b, in_=nm1_d[0:1, :].to_broadcast((P, T)))

        for o in range(DO):
            tmp = pA.tile([P, T], F32, tag="lntmp", bufs=2, name="lntmp")
            nc.vector.tensor_mul(tmp, xb_sb[:, o, :], inv_b)
            nc.vector.tensor_add(h_sb[:, o, :], tmp, nm_b)
        pA.release()

        # ================= QKV projections + RoPE =================
        pWV = tc.alloc_tile_pool(name="pWV", bufs=1)
        pATT = tc.alloc_tile_pool(name="pATT", bufs=1)

        cos_sb = pWV.tile([P, T], BF16, name="cos_sb")
        nc.sync.dma_start(out=cos_sb, in_=cosm[:, :])
        sin_sb = pWV.tile([P, T], BF16, name="sin_sb")
        nc.sync.dma_start(out=sin_sb, in_=sinm[:, :])

        bv_b = pATT.tile([P, D], F32, name="bv_b")
        nc.sync.dma_start(out=bv_b, in_=bvd[0:1, :].to_broadcast((P, D)))
        qT_sb = pATT.tile([P, DO, TQ], BF16, name="qT_sb")
        kT_sb = pATT.tile([P, DO, T], BF16, name="kT_sb")
        v_sb = pATT.tile([P, KT, NH, DK + 1], BF16, name="v_sb")
        nc.vector.memset(v_sb[:, :, :, DK:DK + 1], 1.0)

        # V (row layout [t, dv]) with bias via ones-outer matmul
        for dn in range(2):
            wv_h = pWV.tile([P, DO, 512], BF16, tag="wvh", bufs=1, name="wv_h")
            nc.sync.dma_start(out=wv_h, in_=wvT_r[:, :, dn * 512:(dn + 1) * 512])
            for tt in range(KT):
                psv = psM.tile([P, 512], F32, tag="mm", bufs=2, name="psv")
                for o in range(DO):
                    nc.tensor.matmul(psv, lhsT=h_sb[:, o, tt * P:(tt + 1) * P], rhs=wv_h[:, o, :],
                                     start=(o == 0), stop=(o == DO - 1))
                nc.vector.tensor_tensor(
                    out=v_sb[:, tt, dn * 8:(dn + 1) * 8, 0:DK],
                    in0=psv.rearrange("p (a b) -> p a b", a=8),
                    in1=bv_b[:, dn * 512:(dn + 1) * 512].rearrange("p (a b) -> p a b", a=8),
                    op=OP.add)

        # Q^T and K^T (transposed layout), with rotary applied
        def proj_rot(j, n, w_t, b_col, dst, dst_slice):
            s = slice(n * 512, (n + 1) * 512)
            ps = psM.tile([P, 512], F32, tag="mm", bufs=2, name="psqk")
            for o in range(DO):
                nc.tensor.matmul(ps, lhsT=w_t[:, o, :], rhs=h_sb[:, o, s],
                                 start=(o == 0), stop=(o == DO - 1))
            qa = pATT.tile([P, 512], BF16, tag="qa", bufs=3, name="qa")
            nc.scalar.activation(out=qa, in_=ps, func=AF.Identity, bias=b_col)
            psb = psM.tile([P, 512], F32, tag="mm", bufs=2, name="psjb")
            nc.tensor.matmul(psb, lhsT=jT_sb, rhs=qa, start=True, stop=True)
            rt1 = pATT.tile([P, 512], F32, tag="rt1", bufs=2, name="rt1")
            nc.vector.tensor_mul(rt1, qa, cos_sb[:, s])
            rt2 = pATT.tile([P, 512], F32, tag="rt2", bufs=2, name="rt2")
            nc.vector.tensor_tensor(rt2, psb, sin_sb[:, s], op=OP.mult)
            nc.vector.tensor_add(dst[:, j, dst_slice], rt1, rt2)

        for j in range(DO):
            wq_t = pW.tile([P, DO, P], BF16, tag="wchunk", bufs=3, name="wq_t")
            nc.sync.dma_start(out=wq_t, in_=wqT_r[:, :, j * P:(j + 1) * P])
            for n in range(NQ):
                proj_rot(j, n, wq_t, bq_sb[:, j:j + 1], qT_sb, slice(n * 512, (n + 1) * 512))
            wk_t = pW.tile([P, DO, P], BF16, tag="wchunk", bufs=3, name="wk_t")
            nc.sync.dma_start(out=wk_t, in_=wkT_r[:, :, j * P:(j + 1) * P])
            for n in range(NT):
                proj_rot(j, n, wk_t, bk_sb[:, j:j + 1], kT_sb, slice(n * 512, (n + 1) * 512))

        # ================= attention =================
        for h in range(NH):
            jh, off = h // 2, DK * (h % 2)
            for n in range(NQ):
                qs = slice(n * 512, (n + 1) * 512)
                pso = psM.tile([P, 512], F32, tag="so", bufs=2, name="pso")
                for g in range(KT // 2):
                    sc = psM.tile([P, 2, 512], F32, tag="sc", bufs=2, name="sc")
                    for kk in range(2):
                        kt = g * 2 + kk
                        nc.tensor.matmul(sc[:, kk, :],
                                         lhsT=kT_sb[off:off + DK, jh, kt * P:(kt + 1) * P],
                                         rhs=qT_sb[off:off + DK, jh, qs],
                                         start=True, stop=True)
                    pt = pATT.tile([P, 2, 512], BF16, tag="pt", bufs=3, name="pt")
                    nc.scalar.activation(out=pt, in_=sc, func=AF.Exp, scale=0.125)
                    for kk in range(2):
                        kt = g * 2 + kk
                        nc.tensor.matmul(pso[0:DK + 1, :], lhsT=v_sb[:, kt, h, :], rhs=pt[:, kk, :],
                                         start=(kt == 0), stop=(kt == KT - 1))
                rec = pATT.tile([1, 512], F32, tag="rec", bufs=2, name="rec")
                nc.vector.reciprocal(rec, pso[DK:DK + 1, :])
                rec_d = pDR.tile([1, 512], F32, tag="recd", bufs=4, name="rec_d")
                nc.sync.dma_start(out=rec_d, in_=rec[0:1, :])
                recb = pATT.tile([DK, 512], F32, tag="recb", bufs=2, name="recb")
                nc.sync.dma_start(out=recb, in_=rec_d[0:1, :].to_broadcast((DK, 512)))
                nc.vector.tensor_mul(attnT[off:off + DK, jh, qs], pso[0:DK, :], recb[0:DK, :])
        pATT.release()
        pWV.release()
        pH.release()

        # ================= out-projection + residual =================
        pP = tc.alloc_tile_pool(name="pP", bufs=1)
        y1 = pP.tile([P, DO, TQ], F32, name="y1")
        for j in range(DO):
            wo_t = pW.tile([P, DO, P], BF16, tag="wchunk", bufs=3, name="wo_t")
            nc.sync.dma_start(out=wo_t, in_=woT_r[:, :, j * P:(j + 1) * P])
            xr = pW.tile([P, TQ], F32, tag="xres", bufs=2, name="xr")
            nc.sync.dma_start(out=xr, in_=xT[j * P:(j + 1) * P, 0:TQ])
            for n in range(NQ):
                qs = slice(n * 512, (n + 1) * 512)
                psy = psM.tile([P, 512], F32, tag="mm", bufs=2, name="psy")
                for o in range(DO):
                    nc.tensor.matmul(psy, lhsT=wo_t[:, o, :], rhs=attnT[:, o, qs],
                                     start=(o == 0), stop=(o == DO - 1))
                nc.vector.scalar_tensor_tensor(out=y1[:, j, qs], in0=psy, scalar=bo_sb[:, j:j + 1],
                                               in1=xr[:, qs], op0=OP.add, op1=OP.add)
        # ================= LN2 =================
        pF = tc.alloc_tile_pool(name="pF", bufs=1)
        st2 = psM.tile([P, 2, 512], F32, tag="sc", bufs=2, name="st2")
        st2q = psM.tile([P, 2, 512], F32, tag="sc", bufs=2, name="st2q")
        for o in range(DO):
            y1b = pF.tile([P, TQ], BF16, tag="y1b", bufs=2, name="y1b")
            nc.vector.tensor_copy(out=y1b, in_=y1[:, o, :])
            xsq2 = pF.tile([P, TQ], BF16, tag="xsq2", bufs=2, name="xsq2")
            nc.scalar.activation(out=xsq2, in_=y1[:, o, :], func=AF.Square)
            for n in range(NQ):
                qs = slice(n * 512, (n + 1) * 512)
                nc.tensor.matmul(st2[0:1, n, :], lhsT=ones_bf[:, 0:1], rhs=y1b[:, qs],
                                 start=(o == 0), stop=(o == DO - 1))
                nc.tensor.matmul(st2q[0:1, n, :], lhsT=ones_bf[:, 0:1], rhs=xsq2[:, qs],
                                 start=(o == 0), stop=(o == DO - 1))
        m2_r = pF.tile([1, TQ], F32, tag="r2", bufs=3, name="m2_r")
        e2_r = pF.tile([1, TQ], F32, tag="r2", bufs=3, name="e2_r")
        for n in range(NQ):
            qs = slice(n * 512, (n + 1) * 512)
            nc.vector.tensor_scalar_mul(m2_r[0:1, qs], st2[0:1, n, :], 1.0 / D)
            nc.vector.tensor_scalar_mul(e2_r[0:1, qs], st2q[0:1, n, :], 1.0 / D)
        v2_r = pF.tile([1, TQ], F32, tag="r2", bufs=3, name="v2_r")
        nc.vector.tensor_mul(v2_r, m2_r, m2_r)
        nc.vector.tensor_sub(v2_r, e2_r, v2_r)
        nc.scalar.activation(out=v2_r, in_=v2_r, func=AF.Sqrt, bias=eps_sb[0:1, 0:1])
        i2_r = pF.tile([1, TQ], F32, tag="r2", bufs=3, name="i2_r")
        nc.vector.reciprocal(i2_r, v2_r)
        n2_r = pF.tile([1, TQ], F32, tag="r2", bufs=3, name="n2_r")
        nc.vector.tensor_mul(n2_r, m2_r, i2_r)
        nc.vector.tensor_scalar_mul(n2_r, n2_r, -1.0)
        nc.sync.dma_start(out=inv2_d, in_=i2_r[0:1, :])
        nc.sync.dma_start(out=nm2_d, in_=n2_r[0:1, :])
        inv2_b = pF.tile([P, TQ], F32, name="inv2_b")
        nc.sync.dma_start(out=inv2_b, in_=inv2_d[0:1, :].to_broadcast((P, TQ)))
        nm2_b = pF.tile([P, TQ], F32, name="nm2_b")
        nc.sync.dma_start(out=nm2_b, in_=nm2_d[0:1, :].to_broadcast((P, TQ)))
        h2 = pF.tile([P, DO, TQ], BF16, name="h2")
        for o in range(DO):
            t2 = pF.tile([P, TQ], F32, tag="lntmp2", bufs=1, name="t2")
            nc.vector.tensor_mul(t2, y1[:, o, :], inv2_b)
            nc.vector.tensor_add(h2[:, o, :], t2, nm2_b)

        # ================= FFN =================
        gsb = [pF.tile([P, DFT, 512], BF16, tag="g", bufs=2, name=f"gsb{n}") for n in range(NQ)]
        for f in range(DFT):
            w1_t = pW.tile([P, DO, P], BF16, tag="wchunk", bufs=3, name="w1_t")
            nc.sync.dma_start(out=w1_t, in_=w1T_r[:, :, f * P:(f + 1) * P])
            for n in range(NQ):
                qs = slice(n * 512, (n + 1) * 512)
                psa = psM.tile([P, 512], F32, tag="mm", bufs=2, name="psa")
                for o in range(DO):
                    nc.tensor.matmul(psa, lhsT=w1_t[:, o, :], rhs=h2[:, o, qs],
                                     start=(o == 0), stop=(o == DO - 1))
                nc.scalar.activation(out=gsb[n][:, f, :], in_=psa, func=AF.Gelu,
                                     bias=b1_sb[:, f:f + 1])
        w2T_r = w2T.rearrange("(fo p) d -> p fo d", p=P)
        for j in range(DO):
            psz = [psM.tile([P, 512], F32, tag="so", bufs=2, name=f"psz{n}") for n in range(NQ)]
            for fg in range(DFT // 8):
                w2_t = pW.tile([P, 8, P], BF16, tag="wchunk", bufs=3, name="w2_t")
                nc.sync.dma_start(out=w2_t, in_=w2T_r[:, fg * 8:(fg + 1) * 8, j * P:(j + 1) * P])
                for ff in range(8):
                    f = fg * 8 + ff
                    for n in range(NQ):
                        nc.tensor.matmul(psz[n], lhsT=w2_t[:, ff, :], rhs=gsb[n][:, f, :],
                                         start=(f == 0), stop=(f == DFT - 1))
            for n in range(NQ):
                qs = slice(n * 512, (n + 1) * 512)
                osb = pF.tile([P, 512], F32, tag="osb", bufs=2, name="osb")
                nc.vector.scalar_tensor_tensor(out=osb, in0=psz[n], scalar=b2_sb[:, j:j + 1],
                                               in1=y1[:, j, qs], op0=OP.add, op1=OP.add)
                nc.sync.dma_start(out=outT[j * P:(j + 1) * P, qs], in_=osb)

        pF.release()
        pP.release()
        pH_done = True  # pH released after attention
        psM.release()
        pAT.release()
        pDR.release()
        pW.release()
        pconst.release()

    nc.finalize()
    return nc


def _prep_inputs(inputs):
    f32 = np.float32
    bf16 = ml_dtypes.bfloat16
    x = np.asarray(inputs["x"], dtype=f32)
    Wq = np.asarray(inputs["Wq"], dtype=f32)
    bq = np.asarray(inputs["bq"], dtype=f32)
    Wk = np.asarray(inputs["Wk"], dtype=f32)
    bk = np.asarray(inputs["bk"], dtype=f32)
    Wv = np.asarray(inputs["Wv"], dtype=f32)
    bv = np.asarray(inputs["bv"], dtype=f32)
    Wo = np.asarray(inputs["Wo"], dtype=f32)
    bo = np.asarray(inputs["bo"], dtype=f32)
    W1 = np.asarray(inputs["W1"], dtype=f32)
    b1 = np.asarray(inputs["b1"], dtype=f32)
    W2 = np.asarray(inputs["W2"], dtype=f32)
    b2 = np.asarray(inputs["b2"], dtype=f32)
    g1 = np.asarray(inputs["g1"], dtype=f32)
    be1 = np.asarray(inputs["be1"], dtype=f32)
    g2 = np.asarray(inputs["g2"], dtype=f32)
    be2 = np.asarray(inputs["be2"], dtype=f32)

    # fold LN affines into the consuming projections
    Wq_f, bq_f = Wq * g1[None, :], bq + Wq @ be1
    Wk_f, bk_f = Wk * g1[None, :], bk + Wk @ be1
    Wv_f, bv_f = Wv * g1[None, :], bv + Wv @ be1
    W1_f, b1_f = W1 * g2[None, :], b1 + W1 @ be2

    def t_bf16(w):
        return np.ascontiguousarray(w.T).astype(bf16)

    def pack(b):
        return np.ascontiguousarray(b.reshape(-1, P).T).astype(f32)

    J = np.zeros((P, P), dtype=f32)
    for i in range(0, P, 2):
        J[i, i + 1] = -1.0
        J[i + 1, i] = 1.0

    shared = {
        "wqT": t_bf16(Wq_f), "wkT": t_bf16(Wk_f), "wvT": t_bf16(Wv_f),
        "woT": t_bf16(Wo), "w1T": t_bf16(W1_f), "w2T": t_bf16(W2),
        "jT": np.ascontiguousarray(J.T).astype(bf16),
        "bq": pack(bq_f), "bk": pack(bk_f), "bo": pack(bo), "b2": pack(b2),
        "b1": pack(b1_f), "bv": np.ascontiguousarray(bv_f[None, :]),
    }

    theta = 1.0 / (ROPE_HZ ** (np.arange(0, DK, 2, dtype=np.float64) / DK))  # [32]
    row_idx = (np.arange(P) % DK) // 2

    in_maps = []
    for c in range(N_CORES):
        b, half = c // 2, c % 2
        xTb = x[b].T  # [D, T]
        own = slice(half * TQ, half * TQ + TQ)
        oth = slice((1 - half) * TQ, (1 - half) * TQ + TQ)
        xin = np.ascontiguousarray(np.concatenate([xTb[:, own], xTb[:, oth]], axis=1))
        pos = np.concatenate([np.arange(half * TQ, half * TQ + TQ),
                              np.arange((1 - half) * TQ, (1 - half) * TQ + TQ)])
        ang = theta[:, None] * pos[None, :]  # [32, T]
        cosm = np.ascontiguousarray(np.cos(ang)[row_idx]).astype(bf16)
        sinm = np.ascontiguousarray(np.sin(ang)[row_idx]).astype(bf16)
        m = dict(shared)
        m["xT"] = xin
        m["cosm"] = cosm
        m["sinm"] = sinm
        in_maps.append(m)
    return in_maps


def kernel(**inputs):
    from concourse.bass_utils import run_bass_kernel_spmd

    if "nc" not in _CACHE:
        _CACHE["nc"] = _build_program()
    nc = _CACHE["nc"]
    in_maps = _prep_inputs(inputs)
    res = run_bass_kernel_spmd(nc, in_maps, core_ids=list(range(N_CORES)), **RUN_KWARGS)
    _CACHE["last_results"] = res
    out = np.empty((B, T, D), dtype=np.float32)
    for c in range(N_CORES):
        b, half = c // 2, c % 2
        out[b, half * TQ:half * TQ + TQ, :] = np.asarray(res.results[c]["outT"]).T
    return out
